# revision 21
# baseline (speedup 1.0000x reference)
"""Bilinear sampling (dense_image_warp) Trainium2 kernel — v4.

Strategy (pure data-parallel over batch, 4 samples per NeuronCore):
  out[b,i,j,c] = bilinear_sample(image[b], y=i-256*flow[b,i,j,0],
                                           x=j-256*flow[b,i,j,1])

The image is re-laid out on the host as bf16 with ROW DUPLICATION:
record (r, k) is 256B holding the 2-pixel cell [2k, 2k+1] for BOTH rows
r and r+1, element order [x(2), d(2), c(32)].  A single 512B gather
window (records kk, kk+1, kk = floor(fx/2), idx = fy*128+kk) covers the
whole 2x2 bilinear stencil for all 32 channels -> ONE dma_gather
descriptor per output pixel.  The x-parity is absorbed by 3 "hat"
weights over the 3 leading pixel slots of the 4-pixel window:

  t  = parity(fx) + ax          (in [0,2])
  h0 = relu(1-t); h2 = relu(t-1); h1 = 1-h0-h2
  out = sum_s h_s * ( (1-ay)*top[s] + ay*bot[s] )      s = 0,1,2

v4: the int16 gather-index tiles (wrapped [16, n/16] layout the Q7
ucode wants, replicated for all 8 cores) and the six combined bf16
blend weights are precomputed on the HOST from the flow — they are pure
addressing/weight prep, while all data movement (134MB/core gather) and
the 92M-elem/core blend stay on device.  This empties the device-side
critical path: the kernel is just DMA-in of idx/weight tiles, then a
stream of dma_gather (GPSIMD, 4 SWDGE queues round-robin) + bf16 blend
(DVE) + output DMA.  Each slot's (top,bot) pair is contiguous in the
record, so one tensor_tensor per slot handles both rows (weight tile
holds (1-ay)*h_s and ay*h_s in adjacent halves), then one fold-add sums
the rows.  Output is written bf16, upcast on the host.
"""

import os
import sys

import numpy as np

for _p in ("/opt/trn_rl_repo", "/root/.axon_site/_ro/trn_rl_repo"):
    if os.path.isdir(_p) and _p not in sys.path:
        sys.path.append(_p)

NCORES = 8
B, H, W, C = 32, 256, 256, 32
NS = B // NCORES              # samples per core
NPIX = H * W                  # pixels per sample
NCOLS = NPIX // 128           # 512 "G-layout" columns per sample
NBLK = 16                     # gather blocks per sample
BLKC = NCOLS // NBLK          # 32 G-columns per block
BLKPX = BLKC * 128            # 4096 pixels per block
NUM_IDXS = BLKPX              # gather rows per block (one per pixel)
ELEM = 256                    # gathered bf16 per index (512B window)
STEP = 128                    # index stride in bf16 elems (256B record)
NREC = H * (W // 2)           # records per sample (32768)
SAMPLE_E = NREC * STEP        # bf16 elems per sample image

_CACHE = {}


def _build_module():
    import concourse.bacc as bacc
    import concourse.mybir as mybir
    import concourse.tile as tile
    from concourse import library_config

    bf16 = mybir.dt.bfloat16
    i16 = mybir.dt.int16
    Alu = mybir.AluOpType

    nc = bacc.Bacc(
        "TRN2", target_bir_lowering=False, debug=False, num_swdge_queues=4
    )

    img = nc.dram_tensor("img", [NS * SAMPLE_E + STEP], bf16, kind="ExternalInput")
    idxd = nc.dram_tensor("idxd", [NS, 128, NPIX // 16], i16, kind="ExternalInput")
    gwd = nc.dram_tensor("gwd", [NS, 3, 128, 2 * NCOLS], bf16, kind="ExternalInput")
    out = nc.dram_tensor("out", [NS, 128, NCOLS, C], bf16, kind="ExternalOutput")

    def free_view(ap, offset_elems, dims):
        """View of `ap` keeping its partition dim, replacing free dims."""
        v = ap.copy()
        part = v.ap.to_list()[0]
        v.ap.clear()
        v.ap.extend([part] + [list(d) for d in dims])
        v.offset = v.offset + offset_elems
        return v

    with nc.Block() as _blk:
        @_blk.gpsimd
        def _(g):
            g.load_library(library_config.mlp)

    with tile.TileContext(nc) as tc:
        with (
            tc.tile_pool(name="wts", bufs=NS) as wpool,
            tc.tile_pool(name="idx", bufs=NS) as ipool,
            tc.tile_pool(name="gat", bufs=6) as gpool,
            tc.tile_pool(name="outp", bufs=2) as opool,
            tc.tile_pool(name="tmp", bufs=1) as tpool,
        ):
            V = nc.vector

            # load all samples' index + weight tiles up front (s0 first)
            all_res = []
            for s in range(NS):
                idxw = ipool.tile(
                    [128, NPIX // 16], i16, tag="idxw", name=f"idxw_{s}"
                )
                if s == 0:
                    w0 = BLKPX // 16
                    nc.sync.dma_start(idxw[:, 0:w0], idxd[s, :, 0:w0])
                    nc.sync.dma_start(
                        idxw[:, w0 : NPIX // 16], idxd[s, :, w0 : NPIX // 16]
                    )
                else:
                    nc.sync.dma_start(idxw[:], idxd[s])
                gws = []
                for k in range(3):
                    gw = wpool.tile(
                        [128, 2 * NCOLS], bf16, tag=f"gw{k}", name=f"gw{k}_{s}"
                    )
                    nc.sync.dma_start(gw[:], gwd[s, k])
                    gws.append(gw)
                all_res.append((gws, idxw))

            # per block: gather + blend
            for s in range(NS):
                gws, idxw = all_res[s]
                for blk in range(NBLK):
                    gt_ = gpool.tile([128, BLKC * ELEM], bf16, tag="g", name="g")
                    g3 = gt_[:].rearrange("p (a b) -> p a b", a=BLKC)
                    src = img[:].copy()
                    src.ap.clear()
                    src.ap.extend([[STEP, NREC], [1, ELEM]])
                    src.offset = s * SAMPLE_E
                    idx_ap = idxw[:, blk * (BLKPX // 16) : (blk + 1) * (BLKPX // 16)]
                    nc.gpsimd.dma_gather(
                        g3, src, idx_ap,
                        num_idxs=NUM_IDXS, num_idxs_reg=NUM_IDXS,
                        elem_size=ELEM, elem_step=STEP, single_packet=False,
                        queue_num=blk % 4,
                    )
                    # blend: slot k covers (top,bot) contiguously (64 bf16)
                    ot = tpool.tile([128, BLKC * 64], bf16, tag="ot", name="ot")
                    ta = tpool.tile([128, BLKC * 64], bf16, tag="ta", name="ta")
                    tb = tpool.tile([128, BLKC * 64], bf16, tag="tb", name="tb")

                    def wslice(k):
                        # window slot k: [128, BLKC wins, 2 rows, 32 ch]
                        off = (k // 2) * 128 + (k % 2) * 64
                        return free_view(gt_[:], off, [[ELEM, BLKC], [32, 2], [1, 32]])

                    def wvw(gw):
                        # weight view: [128, BLKC, 2, 32ch-bcast]
                        return free_view(gw[:], blk * BLKC, [[1, BLKC], [NCOLS, 2], [0, 32]])

                    acc3 = lambda t: free_view(t[:], 0, [[64, BLKC], [32, 2], [1, 32]])
                    V.tensor_tensor(out=acc3(ot), in0=wslice(0), in1=wvw(gws[0]), op=Alu.mult)
                    V.tensor_tensor(out=acc3(ta), in0=wslice(1), in1=wvw(gws[1]), op=Alu.mult)
                    V.tensor_tensor(out=acc3(tb), in0=wslice(2), in1=wvw(gws[2]), op=Alu.mult)
                    V.tensor_tensor(out=ot[:], in0=ot[:], in1=ta[:], op=Alu.add)
                    V.tensor_tensor(out=ot[:], in0=ot[:], in1=tb[:], op=Alu.add)
                    # fold top+bot rows
                    fo = opool.tile([128, BLKC * C], bf16, tag="fo", name="fo")
                    fo3 = free_view(fo[:], 0, [[C, BLKC], [1, C]])
                    top_h = free_view(ot[:], 0, [[64, BLKC], [1, 32]])
                    bot_h = free_view(ot[:], 32, [[64, BLKC], [1, 32]])
                    V.tensor_tensor(out=fo3, in0=top_h, in1=bot_h, op=Alu.add)

                    nc.sync.dma_start(
                        out[s, :, blk * BLKC : (blk + 1) * BLKC, :], fo3
                    )

    nc.compile()
    return nc


def _prep_image(image, core):
    import ml_dtypes

    sl = slice(core * NS, (core + 1) * NS)
    img = np.asarray(image[sl], dtype=np.float32)  # [NS,256,256,32]
    r1 = np.concatenate([img[:, 1:], img[:, -1:]], axis=1)  # row r+1, clamped
    a0 = img.reshape(NS, H, W // 2, 2, C)
    a1 = r1.reshape(NS, H, W // 2, 2, C)
    # record layout [s, r, k, x, d, c]
    imgD = np.stack([a0, a1], axis=4).astype(ml_dtypes.bfloat16)
    img_flat = imgD.reshape(-1)
    return np.concatenate([img_flat, np.zeros(STEP, ml_dtypes.bfloat16)])


def _prep_idx_weights(flow, core):
    """Host phase-1: wrapped int16 gather indices + combined bf16 weights.

    Mirrors the reference math in float32.  Returns
      idxd [NS, 128, NPIX//16] int16, gwd [NS, 3, 128, 2*NCOLS] bf16.
    """
    import ml_dtypes

    sl = slice(core * NS, (core + 1) * NS)
    fl = np.asarray(flow[sl], dtype=np.float32).reshape(NS, NPIX, 2)
    m = np.arange(NPIX, dtype=np.int64)
    gi = (m // W).astype(np.float32)  # output row i per pixel
    gj = (m % W).astype(np.float32)   # output col j per pixel

    qy = gi[None, :] - np.float32(IMAGE_SCALE) * fl[:, :, 0]
    qx = gj[None, :] - np.float32(IMAGE_SCALE) * fl[:, :, 1]
    fy = np.clip(np.floor(qy), 0.0, H - 2.0)
    fx = np.clip(np.floor(qx), 0.0, W - 2.0)
    ay = np.clip(qy - fy, 0.0, 1.0).astype(np.float32)
    ax = np.clip(qx - fx, 0.0, 1.0).astype(np.float32)
    kk = np.floor(fx * np.float32(0.5))
    pp = fx - 2.0 * kk
    t = (pp + ax).astype(np.float32)
    h0 = np.maximum(np.float32(1.0) - t, np.float32(0.0))
    h2 = np.maximum(t - np.float32(1.0), np.float32(0.0))
    h1 = np.float32(1.0) - h0 - h2
    ayc = np.float32(1.0) - ay

    idx = (fy * 128.0 + kk).astype(np.int16)  # [NS, NPIX]

    def to_G(v):
        # v [NS, NPIX] -> G-layout [NS, 128, NCOLS]: G[p, c] = v[c*128+p]
        return v.reshape(NS, NCOLS, 128).transpose(0, 2, 1)

    # fold into the wrapped+replicated layout the gather ucode reads:
    # idxw[P, b*256 + j*8 + g] = idx_G[g*16 + P%16, b*32 + j]
    idx_G = to_G(idx)  # [NS, 128, 512]
    P = np.arange(128)
    g = np.arange(8)
    j = np.arange(32)
    b = np.arange(16)
    rows = g[None, None, None, :] * 16 + (P % 16)[:, None, None, None]
    cols = b[None, :, None, None] * 32 + j[None, None, :, None]
    idxd = idx_G[:, rows, cols].reshape(NS, 128, NPIX // 16)

    gwd = np.empty((NS, 3, 128, 2 * NCOLS), np.float32)
    for k, hk in enumerate((h0, h1, h2)):
        gwd[:, k, :, 0:NCOLS] = to_G(hk * ayc)
        gwd[:, k, :, NCOLS:] = to_G(hk * ay)
    return (
        np.ascontiguousarray(idxd),
        np.ascontiguousarray(gwd).astype(ml_dtypes.bfloat16),
    )


IMAGE_SCALE = 256  # reference: flow * image_size


def kernel(image, flow):
    from concourse import bass_utils

    image = np.asarray(image, dtype=np.float32)
    flow = np.asarray(flow, dtype=np.float32)

    if "nc" not in _CACHE:
        _CACHE["nc"] = _build_module()
    nc = _CACHE["nc"]

    in_maps = []
    for core in range(NCORES):
        idxd, gwd = _prep_idx_weights(flow, core)
        in_maps.append(
            {
                "img": _prep_image(image, core),
                "idxd": idxd,
                "gwd": gwd,
            }
        )

    res = bass_utils.run_bass_kernel_spmd(nc, in_maps, core_ids=list(range(NCORES)))

    outs = []
    for r in res.results:
        o = np.asarray(r["out"], dtype=np.float32)
        # [NS, 128, 512, 32]; pixel m = c*128+p at [s, p, c, :]
        outs.append(o.transpose(0, 2, 1, 3).reshape(NS, H, W, C))
    return np.concatenate(outs, axis=0)


# revision 22
# speedup vs baseline: 1.1078x; 1.1078x over previous
"""Bilinear sampling (dense_image_warp) Trainium2 kernel — v4.

Strategy (pure data-parallel over batch, 4 samples per NeuronCore):
  out[b,i,j,c] = bilinear_sample(image[b], y=i-256*flow[b,i,j,0],
                                           x=j-256*flow[b,i,j,1])

The image is re-laid out on the host as bf16 with ROW DUPLICATION:
record (r, k) is 256B holding the 2-pixel cell [2k, 2k+1] for BOTH rows
r and r+1, element order [x(2), d(2), c(32)].  A single 512B gather
window (records kk, kk+1, kk = floor(fx/2), idx = fy*128+kk) covers the
whole 2x2 bilinear stencil for all 32 channels -> ONE dma_gather
descriptor per output pixel.  The x-parity is absorbed by 3 "hat"
weights over the 3 leading pixel slots of the 4-pixel window:

  t  = parity(fx) + ax          (in [0,2])
  h0 = relu(1-t); h2 = relu(t-1); h1 = 1-h0-h2
  out = sum_s h_s * ( (1-ay)*top[s] + ay*bot[s] )      s = 0,1,2

v4: the int16 gather-index tiles (wrapped [16, n/16] layout the Q7
ucode wants, replicated for all 8 cores) and the six combined bf16
blend weights are precomputed on the HOST from the flow — they are pure
addressing/weight prep, while all data movement (134MB/core gather) and
the 92M-elem/core blend stay on device.  This empties the device-side
critical path: the kernel is just DMA-in of idx/weight tiles, then a
stream of dma_gather (GPSIMD, 4 SWDGE queues round-robin) + bf16 blend
(DVE) + output DMA.  Each slot's (top,bot) pair is contiguous in the
record, so one tensor_tensor per slot handles both rows (weight tile
holds (1-ay)*h_s and ay*h_s in adjacent halves), then one fold-add sums
the rows.  Output is written bf16, upcast on the host.
"""

import os
import sys

import numpy as np

for _p in ("/opt/trn_rl_repo", "/root/.axon_site/_ro/trn_rl_repo"):
    if os.path.isdir(_p) and _p not in sys.path:
        sys.path.append(_p)

NCORES = 8
B, H, W, C = 32, 256, 256, 32
NS = B // NCORES              # samples per core
NPIX = H * W                  # pixels per sample
NCOLS = NPIX // 128           # 512 "G-layout" columns per sample
NBLK = 32                     # gather blocks per sample
BLKC = NCOLS // NBLK          # 32 G-columns per block
BLKPX = BLKC * 128            # 4096 pixels per block
NUM_IDXS = BLKPX              # gather rows per block (one per pixel)
ELEM = 256                    # gathered bf16 per index (512B window)
STEP = 128                    # index stride in bf16 elems (256B record)
NREC = H * (W // 2)           # records per sample (32768)
SAMPLE_E = NREC * STEP        # bf16 elems per sample image

_CACHE = {}


def _build_module():
    import concourse.bacc as bacc
    import concourse.mybir as mybir
    import concourse.tile as tile
    from concourse import library_config

    bf16 = mybir.dt.bfloat16
    i16 = mybir.dt.int16
    Alu = mybir.AluOpType

    nc = bacc.Bacc(
        "TRN2", target_bir_lowering=False, debug=False, num_swdge_queues=4
    )

    img = nc.dram_tensor("img", [NS * SAMPLE_E + STEP], bf16, kind="ExternalInput")
    idxd = nc.dram_tensor("idxd", [NS, 128, NPIX // 16], i16, kind="ExternalInput")
    gwd = nc.dram_tensor("gwd", [NS, 3, 128, 2 * NCOLS], bf16, kind="ExternalInput")
    out = nc.dram_tensor("out", [NS, 128, NCOLS, C], bf16, kind="ExternalOutput")

    def free_view(ap, offset_elems, dims):
        """View of `ap` keeping its partition dim, replacing free dims."""
        v = ap.copy()
        part = v.ap.to_list()[0]
        v.ap.clear()
        v.ap.extend([part] + [list(d) for d in dims])
        v.offset = v.offset + offset_elems
        return v

    with nc.Block() as _blk:
        @_blk.gpsimd
        def _(g):
            g.load_library(library_config.mlp)

    with tile.TileContext(nc) as tc:
        with (
            tc.tile_pool(name="wts", bufs=NS) as wpool,
            tc.tile_pool(name="idx", bufs=NS) as ipool,
            tc.tile_pool(name="gat", bufs=6) as gpool,
            tc.tile_pool(name="outp", bufs=2) as opool,
            tc.tile_pool(name="tmp", bufs=1) as tpool,
        ):
            V = nc.vector

            # load all samples' index + weight tiles up front (s0 first)
            all_res = []
            for s in range(NS):
                idxw = ipool.tile(
                    [128, NPIX // 16], i16, tag="idxw", name=f"idxw_{s}"
                )
                if s == 0:
                    w0 = BLKPX // 16
                    nc.sync.dma_start(idxw[:, 0:w0], idxd[s, :, 0:w0])
                    nc.sync.dma_start(
                        idxw[:, w0 : NPIX // 16], idxd[s, :, w0 : NPIX // 16]
                    )
                else:
                    nc.sync.dma_start(idxw[:], idxd[s])
                gws = []
                for k in range(3):
                    gw = wpool.tile(
                        [128, 2 * NCOLS], bf16, tag=f"gw{k}", name=f"gw{k}_{s}"
                    )
                    nc.sync.dma_start(gw[:], gwd[s, k])
                    gws.append(gw)
                all_res.append((gws, idxw))

            # per block: gather + blend
            for s in range(NS):
                gws, idxw = all_res[s]
                for blk in range(NBLK):
                    gt_ = gpool.tile([128, BLKC * ELEM], bf16, tag="g", name="g")
                    g3 = gt_[:].rearrange("p (a b) -> p a b", a=BLKC)
                    src = img[:].copy()
                    src.ap.clear()
                    src.ap.extend([[STEP, NREC], [1, ELEM]])
                    src.offset = s * SAMPLE_E
                    idx_ap = idxw[:, blk * (BLKPX // 16) : (blk + 1) * (BLKPX // 16)]
                    nc.gpsimd.dma_gather(
                        g3, src, idx_ap,
                        num_idxs=NUM_IDXS, num_idxs_reg=NUM_IDXS,
                        elem_size=ELEM, elem_step=STEP, single_packet=False,
                        queue_num=blk % 4,
                    )
                    # blend: slot k covers (top,bot) contiguously (64 bf16)
                    ot = tpool.tile([128, BLKC * 64], bf16, tag="ot", name="ot")
                    ta = tpool.tile([128, BLKC * 64], bf16, tag="ta", name="ta")
                    tb = tpool.tile([128, BLKC * 64], bf16, tag="tb", name="tb")

                    def wslice(k):
                        # window slot k: [128, BLKC wins, 2 rows, 32 ch]
                        off = (k // 2) * 128 + (k % 2) * 64
                        return free_view(gt_[:], off, [[ELEM, BLKC], [32, 2], [1, 32]])

                    def wvw(gw):
                        # weight view: [128, BLKC, 2, 32ch-bcast]
                        return free_view(gw[:], blk * BLKC, [[1, BLKC], [NCOLS, 2], [0, 32]])

                    acc3 = lambda t: free_view(t[:], 0, [[64, BLKC], [32, 2], [1, 32]])
                    V.tensor_tensor(out=acc3(ot), in0=wslice(0), in1=wvw(gws[0]), op=Alu.mult)
                    V.tensor_tensor(out=acc3(ta), in0=wslice(1), in1=wvw(gws[1]), op=Alu.mult)
                    V.tensor_tensor(out=acc3(tb), in0=wslice(2), in1=wvw(gws[2]), op=Alu.mult)
                    V.tensor_tensor(out=ot[:], in0=ot[:], in1=ta[:], op=Alu.add)
                    V.tensor_tensor(out=ot[:], in0=ot[:], in1=tb[:], op=Alu.add)
                    # fold top+bot rows
                    fo = opool.tile([128, BLKC * C], bf16, tag="fo", name="fo")
                    fo3 = free_view(fo[:], 0, [[C, BLKC], [1, C]])
                    top_h = free_view(ot[:], 0, [[64, BLKC], [1, 32]])
                    bot_h = free_view(ot[:], 32, [[64, BLKC], [1, 32]])
                    V.tensor_tensor(out=fo3, in0=top_h, in1=bot_h, op=Alu.add)

                    nc.sync.dma_start(
                        out[s, :, blk * BLKC : (blk + 1) * BLKC, :], fo3
                    )

    nc.compile()
    return nc


def _prep_image(image, core):
    import ml_dtypes

    sl = slice(core * NS, (core + 1) * NS)
    img = np.asarray(image[sl], dtype=np.float32)  # [NS,256,256,32]
    r1 = np.concatenate([img[:, 1:], img[:, -1:]], axis=1)  # row r+1, clamped
    a0 = img.reshape(NS, H, W // 2, 2, C)
    a1 = r1.reshape(NS, H, W // 2, 2, C)
    # record layout [s, r, k, x, d, c]
    imgD = np.stack([a0, a1], axis=4).astype(ml_dtypes.bfloat16)
    img_flat = imgD.reshape(-1)
    return np.concatenate([img_flat, np.zeros(STEP, ml_dtypes.bfloat16)])


def _prep_idx_weights(flow, core):
    """Host phase-1: wrapped int16 gather indices + combined bf16 weights.

    Mirrors the reference math in float32.  Returns
      idxd [NS, 128, NPIX//16] int16, gwd [NS, 3, 128, 2*NCOLS] bf16.
    """
    import ml_dtypes

    sl = slice(core * NS, (core + 1) * NS)
    fl = np.asarray(flow[sl], dtype=np.float32).reshape(NS, NPIX, 2)
    m = np.arange(NPIX, dtype=np.int64)
    gi = (m // W).astype(np.float32)  # output row i per pixel
    gj = (m % W).astype(np.float32)   # output col j per pixel

    qy = gi[None, :] - np.float32(IMAGE_SCALE) * fl[:, :, 0]
    qx = gj[None, :] - np.float32(IMAGE_SCALE) * fl[:, :, 1]
    fy = np.clip(np.floor(qy), 0.0, H - 2.0)
    fx = np.clip(np.floor(qx), 0.0, W - 2.0)
    ay = np.clip(qy - fy, 0.0, 1.0).astype(np.float32)
    ax = np.clip(qx - fx, 0.0, 1.0).astype(np.float32)
    kk = np.floor(fx * np.float32(0.5))
    pp = fx - 2.0 * kk
    t = (pp + ax).astype(np.float32)
    h0 = np.maximum(np.float32(1.0) - t, np.float32(0.0))
    h2 = np.maximum(t - np.float32(1.0), np.float32(0.0))
    h1 = np.float32(1.0) - h0 - h2
    ayc = np.float32(1.0) - ay

    idx = (fy * 128.0 + kk).astype(np.int16)  # [NS, NPIX]

    def to_G(v):
        # v [NS, NPIX] -> G-layout [NS, 128, NCOLS]: G[p, c] = v[c*128+p]
        return v.reshape(NS, NCOLS, 128).transpose(0, 2, 1)

    # fold into the wrapped+replicated layout the gather ucode reads:
    # idxw[P, b*256 + j*8 + g] = idx_G[g*16 + P%16, b*32 + j]
    idx_G = to_G(idx)  # [NS, 128, 512]
    P = np.arange(128)
    g = np.arange(8)
    j = np.arange(32)
    b = np.arange(16)
    rows = g[None, None, None, :] * 16 + (P % 16)[:, None, None, None]
    cols = b[None, :, None, None] * 32 + j[None, None, :, None]
    idxd = idx_G[:, rows, cols].reshape(NS, 128, NPIX // 16)

    gwd = np.empty((NS, 3, 128, 2 * NCOLS), np.float32)
    for k, hk in enumerate((h0, h1, h2)):
        gwd[:, k, :, 0:NCOLS] = to_G(hk * ayc)
        gwd[:, k, :, NCOLS:] = to_G(hk * ay)
    return (
        np.ascontiguousarray(idxd),
        np.ascontiguousarray(gwd).astype(ml_dtypes.bfloat16),
    )


IMAGE_SCALE = 256  # reference: flow * image_size


def kernel(image, flow):
    from concourse import bass_utils

    image = np.asarray(image, dtype=np.float32)
    flow = np.asarray(flow, dtype=np.float32)

    if "nc" not in _CACHE:
        _CACHE["nc"] = _build_module()
    nc = _CACHE["nc"]

    in_maps = []
    for core in range(NCORES):
        idxd, gwd = _prep_idx_weights(flow, core)
        in_maps.append(
            {
                "img": _prep_image(image, core),
                "idxd": idxd,
                "gwd": gwd,
            }
        )

    res = bass_utils.run_bass_kernel_spmd(nc, in_maps, core_ids=list(range(NCORES)))

    outs = []
    for r in res.results:
        o = np.asarray(r["out"], dtype=np.float32)
        # [NS, 128, 512, 32]; pixel m = c*128+p at [s, p, c, :]
        outs.append(o.transpose(0, 2, 1, 3).reshape(NS, H, W, C))
    return np.concatenate(outs, axis=0)


# revision 23
# speedup vs baseline: 1.2946x; 1.1687x over previous
"""Bilinear sampling (dense_image_warp) Trainium2 kernel — v4.

Strategy (pure data-parallel over batch, 4 samples per NeuronCore):
  out[b,i,j,c] = bilinear_sample(image[b], y=i-256*flow[b,i,j,0],
                                           x=j-256*flow[b,i,j,1])

The image is re-laid out on the host as bf16 with ROW DUPLICATION:
record (r, k) is 256B holding the 2-pixel cell [2k, 2k+1] for BOTH rows
r and r+1, element order [x(2), d(2), c(32)].  A single 512B gather
window (records kk, kk+1, kk = floor(fx/2), idx = fy*128+kk) covers the
whole 2x2 bilinear stencil for all 32 channels -> ONE dma_gather
descriptor per output pixel.  The x-parity is absorbed by 3 "hat"
weights over the 3 leading pixel slots of the 4-pixel window:

  t  = parity(fx) + ax          (in [0,2])
  h0 = relu(1-t); h2 = relu(t-1); h1 = 1-h0-h2
  out = sum_s h_s * ( (1-ay)*top[s] + ay*bot[s] )      s = 0,1,2

v4: the int16 gather-index tiles (wrapped [16, n/16] layout the Q7
ucode wants, replicated for all 8 cores) and the six combined bf16
blend weights are precomputed on the HOST from the flow — they are pure
addressing/weight prep, while all data movement (134MB/core gather) and
the 92M-elem/core blend stay on device.  This empties the device-side
critical path: the kernel is just DMA-in of idx/weight tiles, then a
stream of dma_gather (GPSIMD, 4 SWDGE queues round-robin) + bf16 blend
(DVE) + output DMA.  Each slot's (top,bot) pair is contiguous in the
record, so one tensor_tensor per slot handles both rows (weight tile
holds (1-ay)*h_s and ay*h_s in adjacent halves), then one fold-add sums
the rows.  Output is written bf16, upcast on the host.
"""

import os
import sys

import numpy as np

for _p in ("/opt/trn_rl_repo", "/root/.axon_site/_ro/trn_rl_repo"):
    if os.path.isdir(_p) and _p not in sys.path:
        sys.path.append(_p)

NCORES = 8
B, H, W, C = 32, 256, 256, 32
NS = B // NCORES              # samples per core
NPIX = H * W                  # pixels per sample
NCOLS = NPIX // 128           # 512 "G-layout" columns per sample
NBLK = 64                     # gather blocks per sample
BLKC = NCOLS // NBLK          # 32 G-columns per block
BLKPX = BLKC * 128            # 4096 pixels per block
NUM_IDXS = BLKPX              # gather rows per block (one per pixel)
ELEM = 256                    # gathered bf16 per index (512B window)
STEP = 128                    # index stride in bf16 elems (256B record)
NREC = H * (W // 2)           # records per sample (32768)
SAMPLE_E = NREC * STEP        # bf16 elems per sample image

_CACHE = {}


def _build_module():
    import concourse.bacc as bacc
    import concourse.mybir as mybir
    import concourse.tile as tile
    from concourse import library_config

    bf16 = mybir.dt.bfloat16
    i16 = mybir.dt.int16
    Alu = mybir.AluOpType

    nc = bacc.Bacc(
        "TRN2", target_bir_lowering=False, debug=False, num_swdge_queues=4
    )

    img = nc.dram_tensor("img", [NS * SAMPLE_E + STEP], bf16, kind="ExternalInput")
    idxd = nc.dram_tensor("idxd", [NS, 128, NPIX // 16], i16, kind="ExternalInput")
    gwd = nc.dram_tensor("gwd", [NS, 3, 128, 2 * NCOLS], bf16, kind="ExternalInput")
    out = nc.dram_tensor("out", [NS, 128, NCOLS, C], bf16, kind="ExternalOutput")

    def free_view(ap, offset_elems, dims):
        """View of `ap` keeping its partition dim, replacing free dims."""
        v = ap.copy()
        part = v.ap.to_list()[0]
        v.ap.clear()
        v.ap.extend([part] + [list(d) for d in dims])
        v.offset = v.offset + offset_elems
        return v

    with nc.Block() as _blk:
        @_blk.gpsimd
        def _(g):
            g.load_library(library_config.mlp)

    with tile.TileContext(nc) as tc:
        with (
            tc.tile_pool(name="wts", bufs=NS) as wpool,
            tc.tile_pool(name="idx", bufs=NS) as ipool,
            tc.tile_pool(name="gat", bufs=6) as gpool,
            tc.tile_pool(name="outp", bufs=2) as opool,
            tc.tile_pool(name="tmp", bufs=1) as tpool,
        ):
            V = nc.vector

            # load all samples' index + weight tiles up front (s0 first)
            all_res = []
            for s in range(NS):
                idxw = ipool.tile(
                    [128, NPIX // 16], i16, tag="idxw", name=f"idxw_{s}"
                )
                if s == 0:
                    w0 = BLKPX // 16
                    nc.sync.dma_start(idxw[:, 0:w0], idxd[s, :, 0:w0])
                    nc.sync.dma_start(
                        idxw[:, w0 : NPIX // 16], idxd[s, :, w0 : NPIX // 16]
                    )
                else:
                    nc.sync.dma_start(idxw[:], idxd[s])
                gws = []
                for k in range(3):
                    gw = wpool.tile(
                        [128, 2 * NCOLS], bf16, tag=f"gw{k}", name=f"gw{k}_{s}"
                    )
                    nc.sync.dma_start(gw[:], gwd[s, k])
                    gws.append(gw)
                all_res.append((gws, idxw))

            # per pair of gather blocks: 2 gathers into one tile + 1 blend
            PBLKC = 2 * BLKC  # columns per blend unit
            for s in range(NS):
                gws, idxw = all_res[s]
                for pb in range(NBLK // 2):
                    gt_ = gpool.tile([128, PBLKC * ELEM], bf16, tag="g", name="g")
                    for h in range(2):
                        blk = pb * 2 + h
                        dst = free_view(
                            gt_[:], h * BLKC * ELEM, [[ELEM, BLKC], [1, ELEM]]
                        )
                        src = img[:].copy()
                        src.ap.clear()
                        src.ap.extend([[STEP, NREC], [1, ELEM]])
                        src.offset = s * SAMPLE_E
                        idx_ap = idxw[
                            :, blk * (BLKPX // 16) : (blk + 1) * (BLKPX // 16)
                        ]
                        nc.gpsimd.dma_gather(
                            dst, src, idx_ap,
                            num_idxs=NUM_IDXS, num_idxs_reg=NUM_IDXS,
                            elem_size=ELEM, elem_step=STEP, single_packet=False,
                            queue_num=blk % 4,
                        )
                    # blend: slot k covers (top,bot) contiguously (64 bf16)
                    ot = tpool.tile([128, PBLKC * 64], bf16, tag="ot", name="ot")
                    ta = tpool.tile([128, PBLKC * 64], bf16, tag="ta", name="ta")
                    tb = tpool.tile([128, PBLKC * 64], bf16, tag="tb", name="tb")

                    def wslice(k):
                        # window slot k: [128, PBLKC wins, 2 rows, 32 ch]
                        off = (k // 2) * 128 + (k % 2) * 64
                        return free_view(gt_[:], off, [[ELEM, PBLKC], [32, 2], [1, 32]])

                    def wvw(gw):
                        # weight view: [128, PBLKC, 2, 32ch-bcast]
                        return free_view(gw[:], pb * PBLKC, [[1, PBLKC], [NCOLS, 2], [0, 32]])

                    acc3 = lambda t: free_view(t[:], 0, [[64, PBLKC], [32, 2], [1, 32]])
                    V.tensor_tensor(out=acc3(ot), in0=wslice(0), in1=wvw(gws[0]), op=Alu.mult)
                    V.tensor_tensor(out=acc3(ta), in0=wslice(1), in1=wvw(gws[1]), op=Alu.mult)
                    V.tensor_tensor(out=acc3(tb), in0=wslice(2), in1=wvw(gws[2]), op=Alu.mult)
                    V.tensor_tensor(out=ot[:], in0=ot[:], in1=ta[:], op=Alu.add)
                    V.tensor_tensor(out=ot[:], in0=ot[:], in1=tb[:], op=Alu.add)
                    # fold top+bot rows
                    fo = opool.tile([128, PBLKC * C], bf16, tag="fo", name="fo")
                    fo3 = free_view(fo[:], 0, [[C, PBLKC], [1, C]])
                    top_h = free_view(ot[:], 0, [[64, PBLKC], [1, 32]])
                    bot_h = free_view(ot[:], 32, [[64, PBLKC], [1, 32]])
                    V.tensor_tensor(out=fo3, in0=top_h, in1=bot_h, op=Alu.add)

                    nc.sync.dma_start(
                        out[s, :, pb * PBLKC : (pb + 1) * PBLKC, :], fo3
                    )

    nc.compile()
    return nc


def _prep_image(image, core):
    import ml_dtypes

    sl = slice(core * NS, (core + 1) * NS)
    img = np.asarray(image[sl], dtype=np.float32)  # [NS,256,256,32]
    r1 = np.concatenate([img[:, 1:], img[:, -1:]], axis=1)  # row r+1, clamped
    a0 = img.reshape(NS, H, W // 2, 2, C)
    a1 = r1.reshape(NS, H, W // 2, 2, C)
    # record layout [s, r, k, x, d, c]
    imgD = np.stack([a0, a1], axis=4).astype(ml_dtypes.bfloat16)
    img_flat = imgD.reshape(-1)
    return np.concatenate([img_flat, np.zeros(STEP, ml_dtypes.bfloat16)])


def _prep_idx_weights(flow, core):
    """Host phase-1: wrapped int16 gather indices + combined bf16 weights.

    Mirrors the reference math in float32.  Returns
      idxd [NS, 128, NPIX//16] int16, gwd [NS, 3, 128, 2*NCOLS] bf16.
    """
    import ml_dtypes

    sl = slice(core * NS, (core + 1) * NS)
    fl = np.asarray(flow[sl], dtype=np.float32).reshape(NS, NPIX, 2)
    m = np.arange(NPIX, dtype=np.int64)
    gi = (m // W).astype(np.float32)  # output row i per pixel
    gj = (m % W).astype(np.float32)   # output col j per pixel

    qy = gi[None, :] - np.float32(IMAGE_SCALE) * fl[:, :, 0]
    qx = gj[None, :] - np.float32(IMAGE_SCALE) * fl[:, :, 1]
    fy = np.clip(np.floor(qy), 0.0, H - 2.0)
    fx = np.clip(np.floor(qx), 0.0, W - 2.0)
    ay = np.clip(qy - fy, 0.0, 1.0).astype(np.float32)
    ax = np.clip(qx - fx, 0.0, 1.0).astype(np.float32)
    kk = np.floor(fx * np.float32(0.5))
    pp = fx - 2.0 * kk
    t = (pp + ax).astype(np.float32)
    h0 = np.maximum(np.float32(1.0) - t, np.float32(0.0))
    h2 = np.maximum(t - np.float32(1.0), np.float32(0.0))
    h1 = np.float32(1.0) - h0 - h2
    ayc = np.float32(1.0) - ay

    idx = (fy * 128.0 + kk).astype(np.int16)  # [NS, NPIX]

    def to_G(v):
        # v [NS, NPIX] -> G-layout [NS, 128, NCOLS]: G[p, c] = v[c*128+p]
        return v.reshape(NS, NCOLS, 128).transpose(0, 2, 1)

    # fold into the wrapped+replicated layout the gather ucode reads:
    # idxw[P, b*256 + j*8 + g] = idx_G[g*16 + P%16, b*32 + j]
    idx_G = to_G(idx)  # [NS, 128, 512]
    P = np.arange(128)
    g = np.arange(8)
    j = np.arange(32)
    b = np.arange(16)
    rows = g[None, None, None, :] * 16 + (P % 16)[:, None, None, None]
    cols = b[None, :, None, None] * 32 + j[None, None, :, None]
    idxd = idx_G[:, rows, cols].reshape(NS, 128, NPIX // 16)

    gwd = np.empty((NS, 3, 128, 2 * NCOLS), np.float32)
    for k, hk in enumerate((h0, h1, h2)):
        gwd[:, k, :, 0:NCOLS] = to_G(hk * ayc)
        gwd[:, k, :, NCOLS:] = to_G(hk * ay)
    return (
        np.ascontiguousarray(idxd),
        np.ascontiguousarray(gwd).astype(ml_dtypes.bfloat16),
    )


IMAGE_SCALE = 256  # reference: flow * image_size


def kernel(image, flow):
    from concourse import bass_utils

    image = np.asarray(image, dtype=np.float32)
    flow = np.asarray(flow, dtype=np.float32)

    if "nc" not in _CACHE:
        _CACHE["nc"] = _build_module()
    nc = _CACHE["nc"]

    in_maps = []
    for core in range(NCORES):
        idxd, gwd = _prep_idx_weights(flow, core)
        in_maps.append(
            {
                "img": _prep_image(image, core),
                "idxd": idxd,
                "gwd": gwd,
            }
        )

    res = bass_utils.run_bass_kernel_spmd(nc, in_maps, core_ids=list(range(NCORES)))

    outs = []
    for r in res.results:
        o = np.asarray(r["out"], dtype=np.float32)
        # [NS, 128, 512, 32]; pixel m = c*128+p at [s, p, c, :]
        outs.append(o.transpose(0, 2, 1, 3).reshape(NS, H, W, C))
    return np.concatenate(outs, axis=0)
